# revision 2
# baseline (speedup 1.0000x reference)
"""Causal single-head attention (B=4, S=2048, D=1024) on 8 Trainium2 NeuronCores.

Sharding: core = (batch, parity). Each batch's 4 query-chunks of 512 are split
{0,3} / {1,2} across its two cores so causal work balances exactly (each core
processes one 8-k-block chunk and one 16-k-block chunk). K/V are computed from
the full batch sequence on every core (replicated projection — cheaper than
exchanging K/V between cores).

Device algorithm per core (all matmuls bf16 operands, fp32 PSUM accumulation):
  kT[o,s] = WkT.T @ xT      (scores want K transposed: d on partitions)
  v[s,o]  = xT.T @ WvT
  qT[o,s] = (WqT/32).T @ xTq   (softmax scale folded into Wq on host — exact)
  scores_T[sk,sq] = kT_blk.T @ qT_chunk   -> PSUM
  p = exp(scores_T) * mask   (no max-subtraction: logits are O(1) by
                              construction; masked entries multiply to 0)
  out[sq,o] = sum_blk p_blk.T @ v_blk ; den[sq] = sum_blk p_blk.T @ ones
  out = out * (1/den)

The transposed-scores layout keeps the softmax reduction on the PE (ones
matmul) and feeds attn@V without transposing p.
"""

import sys

if "/opt/trn_rl_repo" not in sys.path:
    sys.path.insert(0, "/opt/trn_rl_repo")

import numpy as np
import ml_dtypes

import concourse.mybir as mybir
import concourse.tile as tile
from concourse import bacc
from concourse.bass_utils import run_bass_kernel_spmd

bf16 = ml_dtypes.bfloat16

B, S, D = 4, 2048, 1024
CH = 512            # query-chunk width
BLK = 128           # key-block
NBLK0, NBLK1 = 8, 16  # k-blocks for local chunk 0 / 1
DT8 = D // 128      # contraction tiles
N_CORES = 8
DT_BF = mybir.dt.bfloat16
DT_F32 = mybir.dt.float32

_NC_CACHE = {}


def _emit(tc, xT, xTq, wqT, wkT, wvT, msk, out):
    nc = tc.nc
    Exp = mybir.ActivationFunctionType.Exp

    with (
        tc.tile_pool(name="const", bufs=1) as constp,
        tc.tile_pool(name="kv", bufs=1) as kv,
    ):
        ones = constp.tile([128, 1], DT_BF, tag="ones", name="ones")
        nc.vector.memset(ones, 1.0)

        kT_t = [kv.tile([128, S], DT_BF, tag=f"kT{i}", name=f"kT{i}") for i in range(DT8)]
        v_t = [kv.tile([128, D], DT_BF, tag=f"v{i}", name=f"v{i}") for i in range(S // 128)]
        qT_t = [kv.tile([128, 2 * CH], DT_BF, tag=f"qT{i}", name=f"qT{i}") for i in range(DT8)]

        with (
            tc.tile_pool(name="xw", bufs=1) as xw,
            tc.tile_pool(name="proj_ps", bufs=2, space="PSUM") as pps,
        ):
            xT_t = [xw.tile([128, S], DT_BF, tag=f"xT{i}", name=f"xTs{i}") for i in range(DT8)]
            wk_t = [xw.tile([128, D], DT_BF, tag=f"wk{i}", name=f"wk{i}") for i in range(DT8)]
            wv_t = [xw.tile([128, D], DT_BF, tag=f"wv{i}", name=f"wv{i}") for i in range(DT8)]
            wq_t = [xw.tile([128, D], DT_BF, tag=f"wq{i}", name=f"wq{i}") for i in range(DT8)]
            xq_t = [xw.tile([128, 2 * CH], DT_BF, tag=f"xq{i}", name=f"xq{i}") for i in range(DT8)]

            # DMAs in consumption order so the first matmuls start early.
            for i in range(DT8):
                nc.sync.dma_start(out=wk_t[i], in_=wkT[128 * i : 128 * (i + 1), :])
                nc.sync.dma_start(out=xT_t[i], in_=xT[128 * i : 128 * (i + 1), :])
            for i in range(DT8):
                nc.sync.dma_start(out=wv_t[i], in_=wvT[128 * i : 128 * (i + 1), :])
            for i in range(DT8):
                nc.sync.dma_start(out=wq_t[i], in_=wqT[128 * i : 128 * (i + 1), :])
                nc.sync.dma_start(out=xq_t[i], in_=xTq[128 * i : 128 * (i + 1), :])

            # kT projection: kT[o,s] += WkT[i,o].T @ xT[i,s]
            for ot in range(DT8):
                for sc in range(S // CH):
                    ps = pps.tile([128, CH], DT_F32, tag="pps", name="pps")
                    for i in range(DT8):
                        nc.tensor.matmul(
                            ps,
                            lhsT=wk_t[i][:, 128 * ot : 128 * (ot + 1)],
                            rhs=xT_t[i][:, CH * sc : CH * (sc + 1)],
                            start=(i == 0),
                            stop=(i == DT8 - 1),
                        )
                    nc.vector.tensor_copy(
                        out=kT_t[ot][:, CH * sc : CH * (sc + 1)], in_=ps
                    )
            # v projection: v[s,o] += xT[i,s].T @ WvT[i,o]
            for st in range(S // 128):
                for oc in range(D // CH):
                    ps = pps.tile([128, CH], DT_F32, tag="pps", name="pps")
                    for i in range(DT8):
                        nc.tensor.matmul(
                            ps,
                            lhsT=xT_t[i][:, 128 * st : 128 * (st + 1)],
                            rhs=wv_t[i][:, CH * oc : CH * (oc + 1)],
                            start=(i == 0),
                            stop=(i == DT8 - 1),
                        )
                    nc.scalar.copy(out=v_t[st][:, CH * oc : CH * (oc + 1)], in_=ps)
            # qT projection (Wq pre-scaled by 1/32 on host)
            for ot in range(DT8):
                for sc in range(2):
                    ps = pps.tile([128, CH], DT_F32, tag="pps", name="pps")
                    for i in range(DT8):
                        nc.tensor.matmul(
                            ps,
                            lhsT=wq_t[i][:, 128 * ot : 128 * (ot + 1)],
                            rhs=xq_t[i][:, CH * sc : CH * (sc + 1)],
                            start=(i == 0),
                            stop=(i == DT8 - 1),
                        )
                    nc.scalar.copy(out=qT_t[ot][:, CH * sc : CH * (sc + 1)], in_=ps)

        # ---- attention ----
        with (
            tc.tile_pool(name="attn_sb", bufs=1) as asb,
            tc.tile_pool(name="mask_sb", bufs=4) as msb,
            tc.tile_pool(name="outs_sb", bufs=2) as osb,
            tc.tile_pool(name="score_ps", bufs=2, space="PSUM") as sps,
            tc.tile_pool(name="out_ps", bufs=2, space="PSUM") as ops,
            tc.tile_pool(name="den_ps", bufs=2, space="PSUM") as dps,
        ):
            p_t = {}
            for c, nblk in ((0, NBLK0), (1, NBLK1)):
                for b in range(nblk):
                    ps = sps.tile([128, CH], DT_F32, tag="sps", name="sps")
                    for i in range(DT8):
                        nc.tensor.matmul(
                            ps,
                            lhsT=kT_t[i][:, BLK * b : BLK * (b + 1)],
                            rhs=qT_t[i][:, CH * c : CH * (c + 1)],
                            start=(i == 0),
                            stop=(i == DT8 - 1),
                        )
                    m = msb.tile([128, CH], DT_BF, tag="mask", name="mask")
                    nc.sync.dma_start(out=m, in_=msk[(0 if c == 0 else 8) + b])
                    es = msb.tile([128, CH], DT_BF, tag="es", name="es")
                    nc.scalar.activation(es, ps, Exp)
                    p = asb.tile([128, CH], DT_BF, tag=f"p{c}_{b}", name=f"p{c}_{b}")
                    nc.vector.tensor_mul(p, es, m)
                    p_t[(c, b)] = p

            for c, nblk in ((0, NBLK0), (1, NBLK1)):
                for sqt in range(CH // 128):
                    po = ops.tile([128, D], DT_F32, tag="po", name="po")
                    pd = dps.tile([128, 1], DT_F32, tag="pd", name="pd")
                    for b in range(nblk):
                        pt = p_t[(c, b)][:, 128 * sqt : 128 * (sqt + 1)]
                        nc.tensor.matmul(
                            po[:, 0:CH], lhsT=pt, rhs=v_t[b][:, 0:CH],
                            start=(b == 0), stop=(b == nblk - 1),
                            skip_group_check=True,
                        )
                        nc.tensor.matmul(
                            po[:, CH:D], lhsT=pt, rhs=v_t[b][:, CH:D],
                            start=(b == 0), stop=(b == nblk - 1),
                            skip_group_check=True,
                        )
                        nc.tensor.matmul(
                            pd, lhsT=pt, rhs=ones,
                            start=(b == 0), stop=(b == nblk - 1),
                            skip_group_check=True,
                        )
                    r = osb.tile([128, 1], DT_F32, tag="r", name="r")
                    nc.vector.reciprocal(r, pd)
                    o = osb.tile([128, D], DT_F32, tag="osb", name="osb")
                    nc.vector.tensor_scalar_mul(o, po, r)
                    nc.sync.dma_start(
                        out=out[CH * c + 128 * sqt : CH * c + 128 * (sqt + 1), :],
                        in_=o,
                    )


def build_program():
    nc = bacc.Bacc(
        "TRN2",
        target_bir_lowering=False,
        debug=False,
        enable_asserts=False,
        num_devices=N_CORES,
    )
    xT = nc.dram_tensor("xT", [D, S], DT_BF, kind="ExternalInput").ap()
    xTq = nc.dram_tensor("xTq", [D, 2 * CH], DT_BF, kind="ExternalInput").ap()
    wqT = nc.dram_tensor("wqT", [D, D], DT_BF, kind="ExternalInput").ap()
    wkT = nc.dram_tensor("wkT", [D, D], DT_BF, kind="ExternalInput").ap()
    wvT = nc.dram_tensor("wvT", [D, D], DT_BF, kind="ExternalInput").ap()
    msk = nc.dram_tensor("msk", [NBLK0 + NBLK1, BLK, CH], DT_BF, kind="ExternalInput").ap()
    out = nc.dram_tensor("out", [2 * CH, D], DT_F32, kind="ExternalOutput").ap()
    with tile.TileContext(nc) as tc:
        _emit(tc, xT, xTq, wqT, wkT, wvT, msk, out)
    nc.compile()
    return nc


def get_program():
    if "nc" not in _NC_CACHE:
        _NC_CACHE["nc"] = build_program()
    return _NC_CACHE["nc"]


def _chunks_for(core):
    return (0, 3) if core % 2 == 0 else (1, 2)


def _build_masks(j0, j1):
    """[24,128,512] in {0,1}: rows 0..7 = chunk j0, rows 8..23 = chunk j1.
    allowed(sk=128*blk+p, sq=512*j+c) = sk <= sq."""
    m = np.zeros((NBLK0 + NBLK1, BLK, CH), np.float32)
    p = np.arange(BLK)[:, None]
    c = np.arange(CH)[None, :]
    for b in range(NBLK0):
        m[b] = BLK * b + p <= CH * j0 + c
    for b in range(NBLK1):
        m[NBLK0 + b] = BLK * b + p <= CH * j1 + c
    return m.astype(bf16)


def build_in_maps(x, Wq, Wk, Wv):
    wq = np.ascontiguousarray(Wq.T.astype(np.float32) / 32.0).astype(bf16)
    wk = np.ascontiguousarray(Wk.T).astype(bf16)
    wv = np.ascontiguousarray(Wv.T).astype(bf16)
    masks = {0: _build_masks(0, 3), 1: _build_masks(1, 2)}
    in_maps = []
    for core in range(N_CORES):
        b = core // 2
        j0, j1 = _chunks_for(core)
        xTb = np.ascontiguousarray(x[b].T).astype(bf16)  # [D, S]
        xq = np.ascontiguousarray(
            np.concatenate(
                [xTb[:, j0 * CH : (j0 + 1) * CH], xTb[:, j1 * CH : (j1 + 1) * CH]],
                axis=1,
            )
        )
        in_maps.append(
            {"xT": xTb, "xTq": xq, "wqT": wq, "wkT": wk, "wvT": wv,
             "msk": masks[core % 2]}
        )
    return in_maps


def assemble_output(results):
    out = np.zeros((B, S, D), np.float32)
    for core in range(N_CORES):
        b = core // 2
        j0, j1 = _chunks_for(core)
        o = results[core]["out"]
        out[b, j0 * CH : (j0 + 1) * CH] = o[:CH]
        out[b, j1 * CH : (j1 + 1) * CH] = o[CH:]
    return out


def kernel(x, Wq, Wk, Wv):
    x = np.asarray(x, np.float32)
    nc = get_program()
    in_maps = build_in_maps(x, np.asarray(Wq, np.float32),
                            np.asarray(Wk, np.float32), np.asarray(Wv, np.float32))
    res = run_bass_kernel_spmd(nc, in_maps, core_ids=list(range(N_CORES)))
    return assemble_output(res.results)


# revision 3
# speedup vs baseline: 1.0184x; 1.0184x over previous
"""Causal single-head attention (B=4, S=2048, D=1024) on 8 Trainium2 NeuronCores.

Sharding: core = (batch, parity). Each batch's 4 query-chunks of 512 are split
{0,3} / {1,2} across its two cores so causal work balances exactly (each core
processes one 8-k-block chunk and one 16-k-block chunk). K/V are computed from
the full batch sequence on every core (replicated projection — cheaper than
exchanging K/V between cores).

Device algorithm per core (all matmuls bf16 operands, fp32 PSUM accumulation):
  kT[o,s] = WkT.T @ xT      (scores want K transposed: d on partitions)
  v[s,o]  = xT.T @ WvT
  qT[o,s] = (WqT/32).T @ xTq   (softmax scale folded into Wq on host — exact)
  scores_T[sk,sq] = kT_blk.T @ qT_chunk   -> PSUM
  p = exp(scores_T) * mask   (no max-subtraction: logits are O(1) by
                              construction; masked entries multiply to 0)
  out[sq,o] = sum_blk p_blk.T @ v_blk ; den[sq] = sum_blk p_blk.T @ ones
  out = out * (1/den)

The transposed-scores layout keeps the softmax reduction on the PE (ones
matmul) and feeds attn@V without transposing p.
"""

import sys

if "/opt/trn_rl_repo" not in sys.path:
    sys.path.insert(0, "/opt/trn_rl_repo")

import numpy as np
import ml_dtypes

import concourse.mybir as mybir
import concourse.tile as tile
from concourse import bacc
from concourse.bass_utils import run_bass_kernel_spmd

bf16 = ml_dtypes.bfloat16

B, S, D = 4, 2048, 1024
CH = 512            # query-chunk width
BLK = 128           # key-block
NBLK0, NBLK1 = 8, 16  # k-blocks for local chunk 0 / 1
DT8 = D // 128      # contraction tiles
N_CORES = 8
DT_BF = mybir.dt.bfloat16
DT_F32 = mybir.dt.float32

_NC_CACHE = {}


def _emit(tc, xT, xTq, wqT, wkT, wvT, msk, out):
    nc = tc.nc
    Exp = mybir.ActivationFunctionType.Exp

    with (
        tc.tile_pool(name="const", bufs=1) as constp,
        tc.tile_pool(name="kv", bufs=1) as kv,
    ):
        ones = constp.tile([128, 1], DT_BF, tag="ones", name="ones")
        nc.vector.memset(ones, 1.0)

        kT_t = [kv.tile([128, S], DT_BF, tag=f"kT{i}", name=f"kT{i}") for i in range(DT8)]
        v_t = [kv.tile([128, D], DT_BF, tag=f"v{i}", name=f"v{i}") for i in range(S // 128)]
        qT_t = [kv.tile([128, 2 * CH], DT_BF, tag=f"qT{i}", name=f"qT{i}") for i in range(DT8)]

        with (
            tc.tile_pool(name="xw", bufs=1) as xw,
            tc.tile_pool(name="proj_ps", bufs=2, space="PSUM") as pps,
        ):
            xT_t = [xw.tile([128, S], DT_BF, tag=f"xT{i}", name=f"xTs{i}") for i in range(DT8)]
            wk_t = [xw.tile([128, D], DT_BF, tag=f"wk{i}", name=f"wk{i}") for i in range(DT8)]
            wv_t = [xw.tile([128, D], DT_BF, tag=f"wv{i}", name=f"wv{i}") for i in range(DT8)]
            wq_t = [xw.tile([128, D], DT_BF, tag=f"wq{i}", name=f"wq{i}") for i in range(DT8)]
            xq_t = [xw.tile([128, 2 * CH], DT_BF, tag=f"xq{i}", name=f"xq{i}") for i in range(DT8)]

            # DMAs in consumption order so the first matmuls start early:
            # wv o-chunk 0 (1MB) + xT s-chunk 0 (1MB) unblocks the first
            # v-projection group after ~2MB instead of 6MB.
            for oc in range(D // CH):
                for i in range(DT8):
                    nc.sync.dma_start(
                        out=wv_t[i][:, CH * oc : CH * (oc + 1)],
                        in_=wvT[128 * i : 128 * (i + 1), CH * oc : CH * (oc + 1)],
                    )
                for i in range(DT8):
                    nc.sync.dma_start(
                        out=xT_t[i][:, CH * oc : CH * (oc + 1)],
                        in_=xT[128 * i : 128 * (i + 1), CH * oc : CH * (oc + 1)],
                    )
            for i in range(DT8):
                nc.sync.dma_start(out=wk_t[i], in_=wkT[128 * i : 128 * (i + 1), :])
            for sc in range(2, S // CH):
                for i in range(DT8):
                    nc.sync.dma_start(
                        out=xT_t[i][:, CH * sc : CH * (sc + 1)],
                        in_=xT[128 * i : 128 * (i + 1), CH * sc : CH * (sc + 1)],
                    )
            for i in range(DT8):
                nc.sync.dma_start(out=wq_t[i], in_=wqT[128 * i : 128 * (i + 1), :])
                nc.sync.dma_start(out=xq_t[i], in_=xTq[128 * i : 128 * (i + 1), :])

            # v projection first (st-outer consumes xT column chunks in DMA
            # arrival order): v[s,o] += xT[i,s].T @ WvT[i,o]
            for st in range(S // 128):
                for oc in range(D // CH):
                    ps = pps.tile([128, CH], DT_F32, tag="pps", name="pps")
                    for i in range(DT8):
                        nc.tensor.matmul(
                            ps,
                            lhsT=xT_t[i][:, 128 * st : 128 * (st + 1)],
                            rhs=wv_t[i][:, CH * oc : CH * (oc + 1)],
                            start=(i == 0),
                            stop=(i == DT8 - 1),
                        )
                    nc.scalar.copy(out=v_t[st][:, CH * oc : CH * (oc + 1)], in_=ps)
            # kT projection: kT[o,s] += WkT[i,o].T @ xT[i,s]
            for sc in range(S // CH):
                for ot in range(DT8):
                    ps = pps.tile([128, CH], DT_F32, tag="pps", name="pps")
                    for i in range(DT8):
                        nc.tensor.matmul(
                            ps,
                            lhsT=wk_t[i][:, 128 * ot : 128 * (ot + 1)],
                            rhs=xT_t[i][:, CH * sc : CH * (sc + 1)],
                            start=(i == 0),
                            stop=(i == DT8 - 1),
                        )
                    nc.vector.tensor_copy(
                        out=kT_t[ot][:, CH * sc : CH * (sc + 1)], in_=ps
                    )
            # qT projection (Wq pre-scaled by 1/32 on host)
            for ot in range(DT8):
                for sc in range(2):
                    ps = pps.tile([128, CH], DT_F32, tag="pps", name="pps")
                    for i in range(DT8):
                        nc.tensor.matmul(
                            ps,
                            lhsT=wq_t[i][:, 128 * ot : 128 * (ot + 1)],
                            rhs=xq_t[i][:, CH * sc : CH * (sc + 1)],
                            start=(i == 0),
                            stop=(i == DT8 - 1),
                        )
                    nc.scalar.copy(out=qT_t[ot][:, CH * sc : CH * (sc + 1)], in_=ps)

        # ---- attention ----
        with (
            tc.tile_pool(name="attn_sb", bufs=1) as asb,
            tc.tile_pool(name="mask_sb", bufs=4) as msb,
            tc.tile_pool(name="outs_sb", bufs=2) as osb,
            tc.tile_pool(name="score_ps", bufs=2, space="PSUM") as sps,
            tc.tile_pool(name="out_ps", bufs=2, space="PSUM") as ops,
            tc.tile_pool(name="den_ps", bufs=2, space="PSUM") as dps,
        ):
            p_t = {}
            for c, nblk in ((0, NBLK0), (1, NBLK1)):
                for b in range(nblk):
                    ps = sps.tile([128, CH], DT_F32, tag="sps", name="sps")
                    for i in range(DT8):
                        nc.tensor.matmul(
                            ps,
                            lhsT=kT_t[i][:, BLK * b : BLK * (b + 1)],
                            rhs=qT_t[i][:, CH * c : CH * (c + 1)],
                            start=(i == 0),
                            stop=(i == DT8 - 1),
                        )
                    m = msb.tile([128, CH], DT_BF, tag="mask", name="mask")
                    nc.sync.dma_start(out=m, in_=msk[(0 if c == 0 else 8) + b])
                    es = msb.tile([128, CH], DT_BF, tag="es", name="es")
                    nc.scalar.activation(es, ps, Exp)
                    p = asb.tile([128, CH], DT_BF, tag=f"p{c}_{b}", name=f"p{c}_{b}")
                    nc.vector.tensor_mul(p, es, m)
                    p_t[(c, b)] = p

            for c, nblk in ((0, NBLK0), (1, NBLK1)):
                for sqt in range(CH // 128):
                    po = ops.tile([128, D], DT_F32, tag="po", name="po")
                    pd = dps.tile([128, 1], DT_F32, tag="pd", name="pd")
                    for b in range(nblk):
                        pt = p_t[(c, b)][:, 128 * sqt : 128 * (sqt + 1)]
                        nc.tensor.matmul(
                            po[:, 0:CH], lhsT=pt, rhs=v_t[b][:, 0:CH],
                            start=(b == 0), stop=(b == nblk - 1),
                            skip_group_check=True,
                        )
                        nc.tensor.matmul(
                            po[:, CH:D], lhsT=pt, rhs=v_t[b][:, CH:D],
                            start=(b == 0), stop=(b == nblk - 1),
                            skip_group_check=True,
                        )
                        nc.tensor.matmul(
                            pd, lhsT=pt, rhs=ones,
                            start=(b == 0), stop=(b == nblk - 1),
                            skip_group_check=True,
                        )
                    r = osb.tile([128, 1], DT_F32, tag="r", name="r")
                    nc.vector.reciprocal(r, pd)
                    o = osb.tile([128, D], DT_F32, tag="osb", name="osb")
                    nc.vector.tensor_scalar_mul(o, po, r)
                    nc.sync.dma_start(
                        out=out[CH * c + 128 * sqt : CH * c + 128 * (sqt + 1), :],
                        in_=o,
                    )


def build_program():
    nc = bacc.Bacc(
        "TRN2",
        target_bir_lowering=False,
        debug=False,
        enable_asserts=False,
        num_devices=N_CORES,
    )
    xT = nc.dram_tensor("xT", [D, S], DT_BF, kind="ExternalInput").ap()
    xTq = nc.dram_tensor("xTq", [D, 2 * CH], DT_BF, kind="ExternalInput").ap()
    wqT = nc.dram_tensor("wqT", [D, D], DT_BF, kind="ExternalInput").ap()
    wkT = nc.dram_tensor("wkT", [D, D], DT_BF, kind="ExternalInput").ap()
    wvT = nc.dram_tensor("wvT", [D, D], DT_BF, kind="ExternalInput").ap()
    msk = nc.dram_tensor("msk", [NBLK0 + NBLK1, BLK, CH], DT_BF, kind="ExternalInput").ap()
    out = nc.dram_tensor("out", [2 * CH, D], DT_F32, kind="ExternalOutput").ap()
    with tile.TileContext(nc) as tc:
        _emit(tc, xT, xTq, wqT, wkT, wvT, msk, out)
    nc.compile()
    return nc


def get_program():
    if "nc" not in _NC_CACHE:
        _NC_CACHE["nc"] = build_program()
    return _NC_CACHE["nc"]


def _chunks_for(core):
    return (0, 3) if core % 2 == 0 else (1, 2)


def _build_masks(j0, j1):
    """[24,128,512] in {0,1}: rows 0..7 = chunk j0, rows 8..23 = chunk j1.
    allowed(sk=128*blk+p, sq=512*j+c) = sk <= sq."""
    m = np.zeros((NBLK0 + NBLK1, BLK, CH), np.float32)
    p = np.arange(BLK)[:, None]
    c = np.arange(CH)[None, :]
    for b in range(NBLK0):
        m[b] = BLK * b + p <= CH * j0 + c
    for b in range(NBLK1):
        m[NBLK0 + b] = BLK * b + p <= CH * j1 + c
    return m.astype(bf16)


def build_in_maps(x, Wq, Wk, Wv):
    wq = np.ascontiguousarray(Wq.T.astype(np.float32) / 32.0).astype(bf16)
    wk = np.ascontiguousarray(Wk.T).astype(bf16)
    wv = np.ascontiguousarray(Wv.T).astype(bf16)
    masks = {0: _build_masks(0, 3), 1: _build_masks(1, 2)}
    in_maps = []
    for core in range(N_CORES):
        b = core // 2
        j0, j1 = _chunks_for(core)
        xTb = np.ascontiguousarray(x[b].T).astype(bf16)  # [D, S]
        xq = np.ascontiguousarray(
            np.concatenate(
                [xTb[:, j0 * CH : (j0 + 1) * CH], xTb[:, j1 * CH : (j1 + 1) * CH]],
                axis=1,
            )
        )
        in_maps.append(
            {"xT": xTb, "xTq": xq, "wqT": wq, "wkT": wk, "wvT": wv,
             "msk": masks[core % 2]}
        )
    return in_maps


def assemble_output(results):
    out = np.zeros((B, S, D), np.float32)
    for core in range(N_CORES):
        b = core // 2
        j0, j1 = _chunks_for(core)
        o = results[core]["out"]
        out[b, j0 * CH : (j0 + 1) * CH] = o[:CH]
        out[b, j1 * CH : (j1 + 1) * CH] = o[CH:]
    return out


def kernel(x, Wq, Wk, Wv):
    x = np.asarray(x, np.float32)
    nc = get_program()
    in_maps = build_in_maps(x, np.asarray(Wq, np.float32),
                            np.asarray(Wk, np.float32), np.asarray(Wv, np.float32))
    res = run_bass_kernel_spmd(nc, in_maps, core_ids=list(range(N_CORES)))
    return assemble_output(res.results)
